# revision 1
# baseline (speedup 1.0000x reference)
"""Trainium2 Bass kernel for nn_KLDLoss_18769007083961.

Math reformulation (validated vs reference, rel err ~1e-6):
  For each image b, prototype a with class c(a), define over pixels p:
    s_a[p]  = d_a[p] + (label[p] != c(a)) * (-1e4)      # masked-biased distance
    em_a[p] = exp(s_a[p])                               # exactly 0 off-class (f32 underflow)
    Z_a     = sum_p em_a[p]
    G[a,j]  = sum_p em_a[p] * s_j[p]   (j in same group => same class mask)
    A[a,j]  = G[a,j] / Z_a
  Symmetric KL for a same-group pair (i,j) (log-partition terms cancel):
    kld = 0.5 * (A[j,j] - A[j,i] + A[i,i] - A[i,j])
  loss = mean over valid pairs (class count >= 2) of exp(-kld).

Device kernel (one image per NeuronCore, 8 cores):
  Layout: pixel p = 512*q + 128*w + i  (q = SBUF partition, w = window, i = inner).
  Per window: DMA dist -> s_tile[128, 81*128] (class-major proto order, slot 80 = 1.0),
  DVE builds the -1e4 class bias, ACT computes em = exp(s), then 128 matmuls
  (lhsT = s-slice [128,81], rhs = em-slice [128,80]) accumulate out[m,n] =
  sum_p s_m * em_n into PSUM [81,80]: rows 0..79 = G[n,m]... i.e. out[j,a] = G[a,j],
  row 80 = Z.  Host does the tiny 120-pair combination.
"""

import sys
from contextlib import ExitStack

import numpy as np

sys.path.insert(0, "/opt/trn_rl_repo")

import concourse.bass as bass
import concourse.tile as tile
from concourse import mybir
from concourse.bass_utils import run_bass_kernel_spmd
from concourse.tile import add_dep_helper

B = 8
C = 10
NPROT = 80
P = 65536
Q = 128          # partitions = coarse pixel blocks of 512
W = 4            # windows per image
FI = 128         # inner pixels per window per partition
F32 = mybir.dt.float32

_NC_CACHE = {}


def build_nc():
    nc = bass.Bass()
    # 81 rows: 80 prototypes + a constant-1.0 row that lands in the ones slot
    d_in = nc.dram_tensor("dist", [NPROT + 1, P], F32, kind="ExternalInput")
    # labels [q, 512] packed with the 10 class constants -> cols 512..521
    lab_in = nc.dram_tensor("labcls", [Q, 512 + C], F32, kind="ExternalInput")
    g_out = nc.dram_tensor("g", [81, 80], F32, kind="ExternalOutput")

    with ExitStack() as ctx:
        tc = ctx.enter_context(tile.TileContext(nc))
        singles = ctx.enter_context(tc.tile_pool(name="singles", bufs=1))
        spool = ctx.enter_context(tc.tile_pool(name="spool", bufs=2))
        empool = ctx.enter_context(tc.tile_pool(name="empool", bufs=2))
        mpool = ctx.enter_context(tc.tile_pool(name="mpool", bufs=2))
        psum = ctx.enter_context(tc.tile_pool(name="psum", bufs=1, space="PSUM"))

        labels_t = singles.tile([Q, 512 + C], F32)
        nc.sync.dma_start(out=labels_t, in_=lab_in[:, :])
        cls_t = labels_t[:, 512 : 512 + C]

        g_ps = psum.tile([81, 80], F32)

        # dist[n, p] with p = 512*q + 128*w + i ; natural proto order n = 40*s+4*c+m
        dview = d_in.rearrange("n (q w i) -> n q w i", q=Q, w=W, i=FI)

        first = True
        em_tiles = []
        # Windows 0/1 go to fresh buffers -> plain SP DMAs with no WAR waits.
        # Windows 2/3 recycle buffers; their DMAs are issued from the ACT
        # sequencer right after exp(w-1) (see bottom of the loop), where ACT's
        # clock has already observed the DVE/DMAHW ticks, leaving one PE wait.
        s_tiles = []
        for w in range(2):
            s_w = spool.tile([Q, 81 * FI], F32, tag="s", name=f"s_t{w}")
            nc.sync.dma_start(
                out=s_w.rearrange("p (n i) -> p n i", n=81),
                in_=dview[:, :, w, :].transpose([1, 0, 2]),
            )
            s_tiles.append(s_w)
        for w in range(W):
            s_t = s_tiles[w]

            # mne[p, c, i] = (labels != c) as 1.0/0.0
            mne = mpool.tile([Q, C * FI], F32, tag="mne")
            lab_w = labels_t[:, w * FI : (w + 1) * FI]
            nc.vector.tensor_tensor(
                mne.rearrange("p (c i) -> p c i", c=C),
                lab_w.unsqueeze(1).broadcast_to([Q, C, FI]),
                cls_t.unsqueeze(2).broadcast_to([Q, C, FI]),
                mybir.AluOpType.not_equal,
            )

            # Absorb the dist-DMA completion into DVE's clock with a 1-element
            # copy so the first STT below needs only the mne (DVE) wait.
            probe = mpool.tile([Q, 1], F32, tag="probe", bufs=4)
            nc.vector.tensor_copy(probe, s_t[:, 0:1])
            if w >= 2:
                # Buf recycling gives the first s_t writer WAR deps on both
                # ACT (exp read) and PE (lhsT read) of window w-2.  DVE
                # instructions have a single wait slot, so absorb each dep
                # with its own 1-element op against the old em tile: a read
                # observes ACT, a write observes PE's rhs read.
                em_old = em_tiles[w - 2]
                probe2 = mpool.tile([Q, 1], F32, tag="probe2", bufs=4)
                nc.vector.tensor_copy(probe2, em_old[:, 0:1])
                # disjoint bytes from probe2's read so no same-engine WAR wait
                nc.vector.memset(em_old[:, 1:2], 0.0)

            # s = (mne * -1e4) + d   (in place; walrus caps compute APs at 3 dims,
            # so one op per (scale, class): out [p, 4*FI], in0 [p, m(bcast), i])
            mne_v = mne.rearrange("p (c i) -> p c i", c=C)
            for sc in range(2):
                for c in range(C):
                    n0 = 40 * sc + 4 * c
                    s_dat = s_t[:, n0 * FI : (n0 + 4) * FI]
                    mne_b = mne_v[:, c].unsqueeze(1).broadcast_to([Q, 4, FI])
                    nc.vector.scalar_tensor_tensor(
                        s_dat,
                        mne_b,
                        -1.0e4,
                        s_dat,
                        mybir.AluOpType.mult,
                        mybir.AluOpType.add,
                    )

            # ACT-side absorbers (ACT structs also have one wait slot).  The
            # ones-slot byte is written ONLY by the DMA, so this copy carries
            # just the DMAHW wait.
            dead_act = mpool.tile([Q, 1], F32, tag="dead_act", bufs=4)
            i_abs1 = nc.scalar.copy(dead_act, s_t[:, 80 * FI : 80 * FI + 1])
            act_absorbers = [i_abs1]
            if w >= 2:
                # exp(w) overwrites em(w-2): absorb the WAW-vs-old-exp (ACT
                # sem) by reading an old-em byte, and the WAR-vs-PE-rhs-reads
                # by reading the PSUM accumulator (PE's only visible output).
                dead3 = mpool.tile([Q, 1], F32, tag="dead3", bufs=4)
                if w == 2:
                    src3 = em_tiles[w - 2][:, 2:3]
                else:
                    # reading the previous dead4 absorbs both the old-exp WAW
                    # tick and the PSUM reader-reader serialization tick
                    src3 = last_dead4[0:1, 0:1]
                act_absorbers.append(nc.scalar.copy(dead3[: src3.shape[0]], src3))
                dead4 = mpool.tile([1, 1], F32, tag="dead4", bufs=4)
                act_absorbers.append(nc.scalar.copy(dead4, g_ps[0:1, 0:1]))
                last_dead4 = dead4

            # em = exp(s) (slot 80 -> exp(1), unused by rhs)
            em_t = empool.tile([Q, 81 * FI], F32, tag="em")
            em_tiles.append(em_t)
            i_exp = nc.scalar.activation(em_t, s_t, mybir.ActivationFunctionType.Exp)
            for a in act_absorbers:
                add_dep_helper(i_exp.ins, a.ins, sync=False)

            if w + 1 >= 2 and w + 1 < W:
                s_next = spool.tile([Q, 81 * FI], F32, tag="s", name=f"s_t{w+1}")
                i_dma = nc.scalar.dma_start(
                    out=s_next.rearrange("p (n i) -> p n i", n=81),
                    in_=dview[:, :, w + 1, :].transpose([1, 0, 2]),
                )
                add_dep_helper(i_dma.ins, i_exp.ins, sync=False)
                s_tiles.append(s_next)

            # PE-side absorbers: LDW/MM structs also have a small wait budget,
            # so acquire the DMA then the ACT tick with 1x1 dummy matmuls; the
            # real matmuls then carry only the DVE wait.
            ones_col = s_t[:, 80 * FI : 80 * FI + 1]
            if w == 0:
                dummy_ps = psum.tile([1, 1], F32, tag="dummy", bufs=1)
                dummy_ps2 = psum.tile([1, 1], F32, tag="dummy2", bufs=1)
            i_pabs1 = nc.tensor.matmul(
                dummy_ps, ones_col, ones_col, start=(w == 0), stop=(w == W - 1),
                skip_group_check=True,
            )
            i_pabs2 = nc.tensor.matmul(
                dummy_ps2, ones_col, em_t[:, 0:1], start=(w == 0), stop=(w == W - 1),
                skip_group_check=True,
            )
            add_dep_helper(i_pabs2.ins, i_pabs1.ins, sync=False)

            s_mm = s_t.rearrange("p (n i) -> p n i", n=81)
            em_mm = em_t.rearrange("p (n i) -> p n i", n=81)
            for i in range(FI):
                i_mm = nc.tensor.matmul(
                    g_ps,
                    s_mm[:, :, i],
                    em_mm[:, :80, i],
                    start=first,
                    stop=(w == W - 1 and i == FI - 1),
                )
                if i == 0:
                    add_dep_helper(i_mm.ins, i_pabs2.ins, sync=False)
                first = False

        # DVE absorber for the ACT PSUM-read serialization, so the final
        # PSUM->SBUF copy carries only the PE wait.
        deadf = mpool.tile([1, 1], F32, tag="deadf", bufs=1)
        i_fabs = nc.vector.tensor_copy(deadf, last_dead4)
        g_sb = singles.tile([81, 80], F32)
        i_gcopy = nc.vector.tensor_copy(g_sb, g_ps)
        add_dep_helper(i_gcopy.ins, i_fabs.ins, sync=False)
        nc.sync.dma_start(out=g_out[:, :], in_=g_sb)

    # The kernel-tail drain aggregates every outstanding semaphore into one
    # instruction; the CTRL struct cannot hold that many waits.  Split it
    # into a chain of single-wait drains.
    import copy as _copy

    for fn in nc.m.functions:
        for blk in fn.blocks:
            insts = blk.instructions
            for idx, ins in enumerate(list(insts)):
                si = ins.sync_info
                if type(ins).__name__ == "InstDrain" and si and len(si.on_wait) > 1:
                    waits = list(si.on_wait)
                    si.on_wait = waits[-1:]
                    pos = insts.index(ins)
                    for k, wt in enumerate(waits[:-1]):
                        d2 = _copy.deepcopy(ins)
                        d2.name = f"{ins.name}-split{k}"
                        d2.sync_info = type(si)(on_wait=[wt], on_update=[])
                        insts.insert(pos + k, d2)
                    break

    return nc


def _get_nc():
    if "nc" not in _NC_CACHE:
        _NC_CACHE["nc"] = build_nc()
    return _NC_CACHE["nc"]


def run_device(dist8, labf8, trace=False):
    """dist8: [8, 81, P] f32 permuted + ones row; labf8: [8, P] f32 labels-1."""
    nc = _get_nc()
    cls = np.broadcast_to(np.arange(C, dtype=np.float32)[None, :], (Q, C))
    in_maps = []
    for b in range(B):
        labcls = np.concatenate([labf8[b].reshape(Q, 512), cls], axis=1)
        in_maps.append(
            {"dist": dist8[b], "labcls": np.ascontiguousarray(labcls)}
        )
    return run_bass_kernel_spmd(nc, in_maps, list(range(B)), trace=trace)


def kernel(
    prototype_distances,
    target_labels,
    proto_class,
    pair_i,
    pair_j,
    pair_cls,
    _trace=False,
    _results_out=None,
):
    dist = np.asarray(prototype_distances, dtype=np.float32).reshape(B, NPROT, P)
    labels = np.asarray(target_labels).reshape(B, P).astype(np.int64)
    proto_class = np.asarray(proto_class, dtype=np.int64)
    pair_i = np.asarray(pair_i, dtype=np.int64)
    pair_j = np.asarray(pair_j, dtype=np.int64)
    pair_cls = np.asarray(pair_cls, dtype=np.int64)

    # Permute prototypes so the device's assumed class layout (n%40)//4 holds.
    target_cls = (np.arange(NPROT) % 40) // 4
    perm = np.empty(NPROT, dtype=np.int64)
    for c in range(C):
        protos = np.nonzero(proto_class == c)[0]
        slots = np.nonzero(target_cls == c)[0]
        assert len(protos) == len(slots) == 8, "expect 8 prototypes per class"
        perm[slots] = protos
    inv = np.empty(NPROT, dtype=np.int64)
    inv[perm] = np.arange(NPROT)

    dist_p = np.empty((B, NPROT + 1, P), dtype=np.float32)
    dist_p[:, :NPROT, :] = dist[:, perm, :]
    dist_p[:, NPROT, :] = 1.0
    labf = np.ascontiguousarray((labels - 1).astype(np.float32))

    br = run_device(dist_p, labf, trace=_trace)
    if _results_out is not None:
        _results_out.append(br)

    total_vals = np.float64(0.0)
    total_valid = 0
    for b in range(B):
        out = br.results[b]["g"]  # [81, 80]; out[j, a] = G[a, j], out[80, a] = Z_a
        Z = out[80].astype(np.float64)
        Gt = out[:80].astype(np.float64)  # Gt[j, a] = sum_p em_a * s_j
        with np.errstate(divide="ignore", invalid="ignore"):
            A = np.where(Z[None, :] != 0.0, Gt / Z[None, :], 0.0)  # A[j, a] = E_a[d_j]
        lb = labels[b] - 1
        cnt = np.bincount(lb[lb >= 0], minlength=C)
        ii = inv[pair_i]
        jj = inv[pair_j]
        # A[x, a] = expectation of d_x under softmax of proto a
        kld = 0.5 * (A[jj, jj] - A[jj, ii] + A[ii, ii] - A[ii, jj])
        valid = cnt[pair_cls] >= 2
        total_vals += np.exp(-kld[valid]).sum()
        total_valid += int(valid.sum())

    if total_valid > 0:
        res = np.float32(total_vals / max(total_valid, 1))
    else:
        res = np.float32(0.0)
    return res


if __name__ == "__main__":
    rng = np.random.default_rng(0)
    d = rng.standard_normal((B, NPROT, 256, 256), dtype=np.float32)
    l = rng.integers(0, 11, (B, 256, 256))
    pc = (np.arange(NPROT) % 40) // 4
    pairs = []
    for s in range(2):
        for c in range(C):
            base = s * 40 + c * 4
            for a in range(4):
                for b2 in range(a + 1, 4):
                    pairs.append((base + a, base + b2, c))
    pairs = np.asarray(pairs, np.int32)
    print(kernel(d, l, pc, pairs[:, 0], pairs[:, 1], pairs[:, 2]))



# revision 13
# speedup vs baseline: 1.8082x; 1.8082x over previous
"""Trainium2 Bass kernel for nn_KLDLoss_18769007083961.

Math reformulation (validated vs reference, rel err ~1e-6):
  For each image b, prototype a with class c(a), define over pixels p:
    s_a[p]  = d_a[p] + (label[p] != c(a)) * (-1e4)      # masked-biased distance
    em_a[p] = exp(s_a[p])                               # exactly 0 off-class (underflow)
    Z_a     = sum_p em_a[p]
    G[a,j]  = sum_p em_a[p] * s_j[p]   (j in same class => same mask)
    A[a,j]  = G[a,j] / Z_a
  Symmetric KL for a same-class pair (i,j) (log-partition terms cancel):
    kld = 0.5 * (A[j,j] - A[j,i] + A[i,i] - A[i,j])
  loss = mean over valid pairs (class count >= 2) of exp(-kld).

Device kernel (one image per NeuronCore, 8 cores):
  Pixel p = 512*q + 128*w + i (q = SBUF partition, w = window, i = inner).
  The host pre-packs dist into [W, Q, 80*FI] (proto-major per partition,
  class-major proto permutation) so each window DMA is one contiguous
  41KB run per partition (line-rate HBM).
  Per window: DVE builds the -1e4 class bias into an fp16 s tile
  (10 ops, one per class block of 8 protos), memset writes the ones
  block (slot 80), ACT computes em = exp(s) in fp16, then 128 fp16
  matmuls (lhsT = s-slice [128,81], rhs = em-slice [128,80]) accumulate
  out[m,n] = sum_p s_m * em_n into PSUM [81,80]: out[j,a] = G[a,j],
  row 80 = Z.  Host does the tiny 120-pair combination.
"""

import sys
from contextlib import ExitStack

import numpy as np

sys.path.insert(0, "/opt/trn_rl_repo")

import concourse.bass as bass
import concourse.tile as tile
from concourse import mybir
from concourse.bass_utils import run_bass_kernel_spmd
from concourse.tile import add_dep_helper

B = 8
C = 10
NPROT = 80
P = 65536
Q = 128          # partitions = coarse pixel blocks of 512
W = 4            # windows per image
FI = 128         # inner pixels per window per partition
F32 = mybir.dt.float32
F16 = mybir.dt.float16

# dtype of the dist tensor as uploaded to HBM ("f32" or "f16")
DIST_DT = "f32"
# classes whose bias-STT runs on GPSIMD instead of DVE (load balancing)
GPSIMD_CLASSES = ()

_NC_CACHE = {}


def build_nc():
    nc = bass.Bass()
    dt_in = F32 if DIST_DT == "f32" else F16
    d_in = nc.dram_tensor("dist", [W, Q, NPROT * FI], dt_in, kind="ExternalInput")
    # labels [q, 512] packed with the 10 class constants -> cols 512..521
    lab_in = nc.dram_tensor("labcls", [Q, 512 + C], dt_in, kind="ExternalInput")
    g_out = nc.dram_tensor("g", [NPROT + 1, NPROT], F32, kind="ExternalOutput")

    with ExitStack() as ctx:
        tc = ctx.enter_context(tile.TileContext(nc))
        singles = ctx.enter_context(tc.tile_pool(name="singles", bufs=1))
        dpool = ctx.enter_context(tc.tile_pool(name="dpool", bufs=2))
        spool = ctx.enter_context(tc.tile_pool(name="spool", bufs=2))
        empool = ctx.enter_context(tc.tile_pool(name="empool", bufs=2))
        mpool = ctx.enter_context(tc.tile_pool(name="mpool", bufs=2))
        psum = ctx.enter_context(tc.tile_pool(name="psum", bufs=1, space="PSUM"))

        labels_t = singles.tile([Q, 512 + C], dt_in)
        nc.sync.dma_start(out=labels_t, in_=lab_in[:, :])
        cls_t = labels_t[:, 512 : 512 + C]

        g_ps = psum.tile([NPROT + 1, NPROT], F32)

        first = True
        em_tiles = []
        # Windows 0/1 go to fresh buffers -> plain SP DMAs with no WAR waits.
        # Windows 2/3 recycle buffers; their DMAs are issued from the ACT
        # sequencer right after exp(w-2), whose clock has already observed
        # the DVE ticks of the old buffer's readers, leaving no waits.
        d_tiles = []
        for w in range(2):
            d_t = dpool.tile([Q, NPROT * FI], dt_in, tag="d", name=f"d_t{w}")
            nc.sync.dma_start(out=d_t, in_=d_in[w])
            d_tiles.append(d_t)
        for w in range(W):
            d_t = d_tiles[w]

            # mne[p, c, i] = (labels != c) as 1.0/0.0
            mne = mpool.tile([Q, C * FI], dt_in, tag="mne")
            lab_w = labels_t[:, w * FI : (w + 1) * FI]
            nc.vector.tensor_tensor(
                mne.rearrange("p (c i) -> p c i", c=C),
                lab_w.unsqueeze(1).broadcast_to([Q, C, FI]),
                cls_t.unsqueeze(2).broadcast_to([Q, C, FI]),
                mybir.AluOpType.not_equal,
            )

            # Absorb the d-DMA tick into DVE's clock with a 1-element copy so
            # the STTs below carry no DMA wait.
            probe = mpool.tile([Q, 1], dt_in, tag="probe", bufs=4)
            nc.vector.tensor_copy(probe, d_t[:, 0:1])
            if w >= 2:
                # Buffer recycling gives this window's s/em writers WAR deps
                # on ACT (exp read of s(w-2)) and PE (matmul reads of s/em
                # (w-2)).  DVE ops have one wait slot, so absorb each with
                # its own 1-element read: em_old[0:1] observes the ACT exp
                # tick; dummy_ps1 (written by i_pabs1(w-1), which follows all
                # window w-2 matmuls in PE order) observes the PE reads.
                em_old = em_tiles[w - 2]
                probe2 = mpool.tile([Q, 1], dt_in, tag="probe2", bufs=4)
                nc.vector.tensor_copy(probe2, em_old[:, 0:1])
                probe3 = mpool.tile([1, 1], F32, tag="probe3", bufs=4)
                nc.vector.tensor_copy(probe3, dummy_ps1[0:1, 0:1])

            # s = (mne * -1e4) + d, fp16, one op per class block of 8 protos
            s_t = spool.tile([Q, (NPROT + 1) * FI], F16, tag="s")
            mne_v = mne.rearrange("p (c i) -> p c i", c=C)
            d_v = d_t.rearrange("p (n i) -> p n i", n=NPROT)
            s_v = s_t.rearrange("p (n i) -> p n i", n=NPROT + 1)
            for c in range(C):
                eng = nc.gpsimd if c in GPSIMD_CLASSES else nc.vector
                eng.scalar_tensor_tensor(
                    s_v[:, 8 * c : 8 * (c + 1), :],
                    mne_v[:, c].unsqueeze(1).broadcast_to([Q, 8, FI]),
                    -1.0e4,
                    d_v[:, 8 * c : 8 * (c + 1), :],
                    mybir.AluOpType.mult,
                    mybir.AluOpType.add,
                )
            # ones block (slot 80) feeds the Z row of the matmul
            nc.vector.memset(s_t[:, NPROT * FI :], 1.0)

            # ACT-side absorber: exp(w) overwrites em(w-2), which PE read as
            # rhs; a 1-element scalar-engine read of dummy_ps2 (last written
            # by i_pabs2(w-1), after all w-2 matmuls) carries that PE wait so
            # the exp itself only waits on DVE.
            act_absorbers = []
            if w >= 2:
                dead_act = mpool.tile([1, 1], F32, tag="dead_act", bufs=4)
                act_absorbers.append(nc.scalar.copy(dead_act, dummy_ps2[0:1, 0:1]))
                # absorb the DVE tick (s ready) with a 1-element read of the
                # ones column so exp(w) itself carries only its WAW self-wait
                dead_act3 = mpool.tile([Q, 1], F16, tag="dead_act3", bufs=4)
                act_absorbers.append(
                    nc.scalar.copy(dead_act3, s_t[:, NPROT * FI : NPROT * FI + 1])
                )

            # em = exp(s) over the 80 real proto blocks
            em_t = empool.tile([Q, NPROT * FI], F16, tag="em")
            em_tiles.append(em_t)
            i_exp = nc.scalar.activation(
                em_t, s_t[:, : NPROT * FI], mybir.ActivationFunctionType.Exp
            )
            for a in act_absorbers:
                add_dep_helper(i_exp.ins, a.ins, sync=False)
            if w > 0:
                # keep exps in ACT program order so the WAW over the
                # recycled em buffer needs no explicit self-wait
                add_dep_helper(i_exp.ins, prev_exp.ins, sync=False)
            prev_exp = i_exp

            if w + 2 < W:
                d_next = dpool.tile(
                    [Q, NPROT * FI], dt_in, tag="d", name=f"d_t{w+2}"
                )
                i_dma = nc.scalar.dma_start(out=d_next, in_=d_in[w + 2])
                add_dep_helper(i_dma.ins, i_exp.ins, sync=False)
                d_tiles.append(d_next)

            # PE-side absorbers: 1x1 dummy matmuls acquire the DVE tick
            # (ones col of s) and the ACT tick (em) so the real matmuls
            # carry no waits.  Their dummy PSUM cells double as the
            # "window w matmuls issued" markers the w+1 absorbers read.
            ones_col = s_t[:, NPROT * FI : NPROT * FI + 1]
            if w == 0:
                dummy_ps1 = psum.tile([1, 1], F32, tag="dummy1", bufs=1)
                dummy_ps2 = psum.tile([1, 1], F32, tag="dummy2", bufs=1)
            i_pabs1 = nc.tensor.matmul(
                dummy_ps1, ones_col, ones_col,
                start=(w == 0), stop=(w == W - 1),
                skip_group_check=True,
            )
            i_pabs2 = nc.tensor.matmul(
                dummy_ps2, ones_col, em_t[:, 0:1],
                start=(w == 0), stop=(w == W - 1),
                skip_group_check=True,
            )
            add_dep_helper(i_pabs2.ins, i_pabs1.ins, sync=False)

            s_mm = s_t.rearrange("p (n i) -> p n i", n=NPROT + 1)
            em_mm = em_t.rearrange("p (n i) -> p n i", n=NPROT)
            for i in range(FI):
                i_mm = nc.tensor.matmul(
                    g_ps,
                    s_mm[:, :, i],
                    em_mm[:, :, i],
                    start=first,
                    stop=(w == W - 1 and i == FI - 1),
                )
                if i == 0:
                    add_dep_helper(i_mm.ins, i_pabs2.ins, sync=False)
                first = False

        g_sb = singles.tile([NPROT + 1, NPROT], F32)
        nc.vector.tensor_copy(g_sb, g_ps)
        nc.sync.dma_start(out=g_out[:, :], in_=g_sb)

    # Hardware instruction structs hold only one sync wait.  Move any excess
    # waits onto single-wait InstDrains injected just before the instruction
    # on the same engine queue (the union of waits still precedes execution).
    import copy as _copy

    drain_tmpl = {}
    for fn in nc.m.functions:
        for blk in fn.blocks:
            for ins in blk.instructions:
                if type(ins).__name__ == "InstDrain" and ins.engine is not None:
                    drain_tmpl.setdefault(ins.engine, ins)

    seq = [0]

    def _drain_clone(engine, wait):
        tmpl = drain_tmpl[engine]
        d2 = _copy.deepcopy(tmpl)
        seq[0] += 1
        d2.name = f"waitsplit-{seq[0]}"
        d2.sync_info = type(tmpl.sync_info)(on_wait=[wait], on_update=[])
        return d2

    for fn in nc.m.functions:
        for blk in fn.blocks:
            insts = blk.instructions
            idx = 0
            while idx < len(insts):
                ins = insts[idx]
                si = ins.sync_info
                if si and len(si.on_wait) > 1 and ins.engine in drain_tmpl:
                    waits = list(si.on_wait)
                    si.on_wait = waits[-1:]
                    for k, wt in enumerate(waits[:-1]):
                        insts.insert(idx + k, _drain_clone(ins.engine, wt))
                    idx += len(waits) - 1
                idx += 1

    return nc


def _get_nc():
    if "nc" not in _NC_CACHE:
        _NC_CACHE["nc"] = build_nc()
    return _NC_CACHE["nc"]


def run_device(dist8, labf8, trace=False):
    """dist8: [8, W, Q, 80*FI]; labf8: [8, P] labels-1 as float."""
    nc = _get_nc()
    np_dt = np.float32 if DIST_DT == "f32" else np.float16
    cls = np.broadcast_to(np.arange(C, dtype=np_dt)[None, :], (Q, C))
    in_maps = []
    for b in range(B):
        labcls = np.concatenate([labf8[b].reshape(Q, 512).astype(np_dt), cls], axis=1)
        in_maps.append(
            {"dist": dist8[b], "labcls": np.ascontiguousarray(labcls)}
        )
    return run_bass_kernel_spmd(nc, in_maps, list(range(B)), trace=trace)


def kernel(
    prototype_distances,
    target_labels,
    proto_class,
    pair_i,
    pair_j,
    pair_cls,
    _trace=False,
    _results_out=None,
):
    dist = np.asarray(prototype_distances, dtype=np.float32).reshape(B, NPROT, P)
    labels = np.asarray(target_labels).reshape(B, P).astype(np.int64)
    proto_class = np.asarray(proto_class, dtype=np.int64)
    pair_i = np.asarray(pair_i, dtype=np.int64)
    pair_j = np.asarray(pair_j, dtype=np.int64)
    pair_cls = np.asarray(pair_cls, dtype=np.int64)

    # Permute prototypes class-major: slot n holds a prototype of class n//8.
    perm = np.empty(NPROT, dtype=np.int64)
    for c in range(C):
        protos = np.nonzero(proto_class == c)[0]
        assert len(protos) == 8, "expect 8 prototypes per class"
        perm[8 * c : 8 * (c + 1)] = protos
    inv = np.empty(NPROT, dtype=np.int64)
    inv[perm] = np.arange(NPROT)

    # Pack into the device DMA layout [B, W, Q, n, i]: pixel p = 512q+128w+i.
    np_dt = np.float32 if DIST_DT == "f32" else np.float16
    dist_p = np.ascontiguousarray(
        dist[:, perm, :]
        .reshape(B, NPROT, Q, W, FI)
        .transpose(0, 3, 2, 1, 4)
        .reshape(B, W, Q, NPROT * FI)
        .astype(np_dt)
    )
    labf = np.ascontiguousarray((labels - 1).astype(np_dt))

    br = run_device(dist_p, labf, trace=_trace)
    if _results_out is not None:
        _results_out.append(br)

    total_vals = np.float64(0.0)
    total_valid = 0
    for b in range(B):
        out = br.results[b]["g"]  # [81, 80]; out[j, a] = G[a, j], out[80, a] = Z_a
        Z = out[NPROT].astype(np.float64)
        Gt = out[:NPROT].astype(np.float64)  # Gt[j, a] = sum_p em_a * s_j
        with np.errstate(divide="ignore", invalid="ignore"):
            A = np.where(Z[None, :] != 0.0, Gt / Z[None, :], 0.0)  # A[j, a] = E_a[d_j]
        lb = labels[b] - 1
        cnt = np.bincount(lb[lb >= 0], minlength=C)
        ii = inv[pair_i]
        jj = inv[pair_j]
        # A[x, a] = expectation of d_x under softmax of proto a
        kld = 0.5 * (A[jj, jj] - A[jj, ii] + A[ii, ii] - A[ii, jj])
        valid = cnt[pair_cls] >= 2
        total_vals += np.exp(-kld[valid]).sum()
        total_valid += int(valid.sum())

    if total_valid > 0:
        res = np.float32(total_vals / max(total_valid, 1))
    else:
        res = np.float32(0.0)
    return res


if __name__ == "__main__":
    rng = np.random.default_rng(0)
    d = rng.standard_normal((B, NPROT, 256, 256), dtype=np.float32)
    l = rng.integers(0, 11, (B, 256, 256))
    pc = (np.arange(NPROT) % 40) // 4
    pairs = []
    for s in range(2):
        for c in range(C):
            base = s * 40 + c * 4
            for a in range(4):
                for b2 in range(a + 1, 4):
                    pairs.append((base + a, base + b2, c))
    pairs = np.asarray(pairs, np.int32)
    print(kernel(d, l, pc, pairs[:, 0], pairs[:, 1], pairs[:, 2]))


# revision 14
# speedup vs baseline: 2.3347x; 1.2911x over previous
"""Trainium2 Bass kernel for nn_KLDLoss_18769007083961.

Math reformulation (validated vs reference, rel err ~1e-6):
  For each image b, prototype a with class c(a), define over pixels p:
    s_a[p]  = d_a[p] + (label[p] != c(a)) * (-1e4)      # masked-biased distance
    em_a[p] = exp(s_a[p])                               # exactly 0 off-class (underflow)
    Z_a     = sum_p em_a[p]
    G[a,j]  = sum_p em_a[p] * s_j[p]   (j in same class => same mask)
    A[a,j]  = G[a,j] / Z_a
  Symmetric KL for a same-class pair (i,j) (log-partition terms cancel):
    kld = 0.5 * (A[j,j] - A[j,i] + A[i,i] - A[i,j])
  loss = mean over valid pairs (class count >= 2) of exp(-kld).

Device kernel (one image per NeuronCore, 8 cores):
  Pixel p = 512*q + 64*w + i (q = SBUF partition, w = window, i = inner).
  The host pre-packs dist into [W, Q, FI*80] (i-major: per partition line,
  FI pixel-slots of 80 protos each, class-major proto permutation) so each
  window DMA is one contiguous run per partition (line-rate HBM) and each
  matmul operand slice is contiguous in SBUF.
  Per window: DVE builds the -1e4 class bias into an fp16 s tile (10 STT
  ops, one per class block of 8 protos; s has 81 slots per pixel, slot 80
  memset to 1.0 for the Z row), ACT computes em = exp(s) in fp16, then FI
  fp16 matmuls (lhsT = s-slice [128,81] contiguous, rhs = em-slice
  [128,80] contiguous) accumulate out[m,n] = sum_p s_m * em_n into PSUM
  [81,80]: out[j,a] = G[a,j], row 80 = Z.  Host does the tiny 120-pair
  combination.
"""

import sys
from contextlib import ExitStack

import numpy as np

sys.path.insert(0, "/opt/trn_rl_repo")

import concourse.bass as bass
import concourse.tile as tile
from concourse import mybir
from concourse.bass_utils import run_bass_kernel_spmd
from concourse.tile import add_dep_helper

B = 8
C = 10
NPROT = 80
P = 65536
Q = 128          # partitions = coarse pixel blocks of 512
W = 8            # windows per image
FI = 512 // W    # inner pixels per window per partition
F32 = mybir.dt.float32
F16 = mybir.dt.float16

# dtype of the dist tensor as uploaded to HBM ("f32" or "f16")
DIST_DT = "f32"
# classes whose bias-STT runs on GPSIMD instead of DVE (load balancing)
GPSIMD_CLASSES = ()

_NC_CACHE = {}


def build_nc():
    nc = bass.Bass()
    dt_in = F32 if DIST_DT == "f32" else F16
    d_in = nc.dram_tensor("dist", [W, Q, FI * NPROT], dt_in, kind="ExternalInput")
    # labels [q, 512] packed with the 10 class constants -> cols 512..521
    lab_in = nc.dram_tensor("labcls", [Q, 512 + C], dt_in, kind="ExternalInput")
    g_out = nc.dram_tensor("g", [NPROT + 1, NPROT], F32, kind="ExternalOutput")

    with ExitStack() as ctx:
        tc = ctx.enter_context(tile.TileContext(nc))
        singles = ctx.enter_context(tc.tile_pool(name="singles", bufs=1))
        dpool = ctx.enter_context(tc.tile_pool(name="dpool", bufs=3))
        spool = ctx.enter_context(tc.tile_pool(name="spool", bufs=2))
        empool = ctx.enter_context(tc.tile_pool(name="empool", bufs=2))
        mpool = ctx.enter_context(tc.tile_pool(name="mpool", bufs=2))
        psum = ctx.enter_context(tc.tile_pool(name="psum", bufs=1, space="PSUM"))

        labels_t = singles.tile([Q, 512 + C], dt_in)
        nc.sync.dma_start(out=labels_t, in_=lab_in[:, :])
        cls_t = labels_t[:, 512 : 512 + C]

        g_ps = psum.tile([NPROT + 1, NPROT], F32)

        first = True
        em_tiles = []
        exps = []
        # The first dpool.bufs windows go to fresh buffers -> plain SP DMAs
        # with no WAR waits.  Later windows recycle buffers; their DMAs are
        # issued from the ACT sequencer right after exp(w - bufs), whose
        # clock has already observed the DVE ticks of the old buffer's
        # readers, leaving no waits.
        DB = 3  # dpool bufs
        d_tiles = []
        for w in range(DB):
            d_t = dpool.tile([Q, FI * NPROT], dt_in, tag="d", name=f"d_t{w}")
            nc.sync.dma_start(out=d_t, in_=d_in[w])
            d_tiles.append(d_t)
        for w in range(W):
            d_t = d_tiles[w]

            # mne[p, c, i] = (labels != c) as 1.0/0.0
            mne = mpool.tile([Q, C * FI], dt_in, tag="mne")
            lab_w = labels_t[:, w * FI : (w + 1) * FI]
            nc.vector.tensor_tensor(
                mne.rearrange("p (c i) -> p c i", c=C),
                lab_w.unsqueeze(1).broadcast_to([Q, C, FI]),
                cls_t.unsqueeze(2).broadcast_to([Q, C, FI]),
                mybir.AluOpType.not_equal,
            )

            # Absorb the d-DMA tick into DVE's clock with a 1-element copy so
            # the STTs below carry no DMA wait.
            probe = mpool.tile([Q, 1], dt_in, tag="probe", bufs=4)
            nc.vector.tensor_copy(probe, d_t[:, 0:1])
            if w >= 2:
                # s/em buffer recycling gives this window's writers WAR deps
                # on ACT (exp read of s(w-2)) and PE (matmul reads of s/em
                # (w-2)).  Absorb each with a 1-element read: em_old[0:1]
                # observes the ACT exp tick; dummy_ps1 (written by
                # i_pabs1(w-1), after all w-2 matmuls in PE order) observes
                # the PE reads.
                em_old = em_tiles[w - 2]
                probe2 = mpool.tile([Q, 1], dt_in, tag="probe2", bufs=4)
                nc.vector.tensor_copy(probe2, em_old[:, 0:1])
                probe3 = mpool.tile([1, 1], F32, tag="probe3", bufs=4)
                nc.vector.tensor_copy(probe3, dummy_ps1[0:1, 0:1])

            # s[p, i, n] = (mne * -1e4) + d, fp16, one op per class block
            s_t = spool.tile([Q, FI * (NPROT + 1)], F16, tag="s")
            mne_v = mne.rearrange("p (c i) -> p c i", c=C)
            d_v = d_t.rearrange("p (i n) -> p i n", n=NPROT)
            s_v = s_t.rearrange("p (i n) -> p i n", n=NPROT + 1)
            for c in range(C):
                eng = nc.gpsimd if c in GPSIMD_CLASSES else nc.vector
                eng.scalar_tensor_tensor(
                    s_v[:, :, 8 * c : 8 * (c + 1)],
                    mne_v[:, c].unsqueeze(2).broadcast_to([Q, FI, 8]),
                    -1.0e4,
                    d_v[:, :, 8 * c : 8 * (c + 1)],
                    mybir.AluOpType.mult,
                    mybir.AluOpType.add,
                )
            # ones slot (n = 80 of each pixel block) feeds the Z row
            nc.vector.memset(s_v[:, :, NPROT : NPROT + 1], 1.0)

            # ACT-side absorbers so exp(w) carries a single wait: dummy_ps2
            # (written by i_pabs2(w-1), after all w-2 matmuls) observes the
            # PE reads of em(w-2); the ones column observes the DVE tick.
            act_absorbers = []
            if w >= 2:
                dead_act = mpool.tile([1, 1], F32, tag="dead_act", bufs=4)
                act_absorbers.append(nc.scalar.copy(dead_act, dummy_ps2[0:1, 0:1]))
                dead_act3 = mpool.tile([Q, 1], F16, tag="dead_act3", bufs=4)
                act_absorbers.append(
                    nc.scalar.copy(dead_act3, s_t[:, NPROT : NPROT + 1])
                )

            # em = exp(s), including the ones slot (exp(1) unused by rhs)
            em_t = empool.tile([Q, FI * (NPROT + 1)], F16, tag="em")
            em_tiles.append(em_t)
            i_exp = nc.scalar.activation(
                em_t, s_t, mybir.ActivationFunctionType.Exp
            )
            for a in act_absorbers:
                add_dep_helper(i_exp.ins, a.ins, sync=False)
            if w > 0:
                add_dep_helper(i_exp.ins, exps[-1].ins, sync=False)
            exps.append(i_exp)

            if w + DB < W:
                d_next = dpool.tile(
                    [Q, FI * NPROT], dt_in, tag="d", name=f"d_t{w+DB}"
                )
                i_dma = nc.scalar.dma_start(out=d_next, in_=d_in[w + DB])
                add_dep_helper(i_dma.ins, i_exp.ins, sync=False)
                d_tiles.append(d_next)

            # PE-side absorbers: 1x1 dummy matmuls acquire the DVE tick
            # (ones col of s) and the ACT tick (em) so the real matmuls
            # carry no waits.  Their dummy PSUM cells double as the
            # "window w matmuls issued" markers the w+1 absorbers read.
            ones_col = s_t[:, NPROT : NPROT + 1]
            if w == 0:
                dummy_ps1 = psum.tile([1, 1], F32, tag="dummy1", bufs=1)
                dummy_ps2 = psum.tile([1, 1], F32, tag="dummy2", bufs=1)
            i_pabs1 = nc.tensor.matmul(
                dummy_ps1, ones_col, ones_col,
                start=(w == 0), stop=(w == W - 1),
                skip_group_check=True,
            )
            i_pabs2 = nc.tensor.matmul(
                dummy_ps2, ones_col, em_t[:, 0:1],
                start=(w == 0), stop=(w == W - 1),
                skip_group_check=True,
            )
            add_dep_helper(i_pabs2.ins, i_pabs1.ins, sync=False)

            for i in range(FI):
                i_mm = nc.tensor.matmul(
                    g_ps,
                    s_t[:, i * (NPROT + 1) : (i + 1) * (NPROT + 1)],
                    em_t[:, i * (NPROT + 1) : i * (NPROT + 1) + NPROT],
                    start=first,
                    stop=(w == W - 1 and i == FI - 1),
                )
                if i == 0:
                    add_dep_helper(i_mm.ins, i_pabs2.ins, sync=False)
                first = False

        g_sb = singles.tile([NPROT + 1, NPROT], F32)
        nc.vector.tensor_copy(g_sb, g_ps)
        nc.sync.dma_start(out=g_out[:, :], in_=g_sb)

    # Hardware instruction structs hold only one sync wait.  Move any excess
    # waits onto single-wait InstDrains injected just before the instruction
    # on the same engine queue (the union of waits still precedes execution).
    import copy as _copy

    drain_tmpl = {}
    for fn in nc.m.functions:
        for blk in fn.blocks:
            for ins in blk.instructions:
                if type(ins).__name__ == "InstDrain" and ins.engine is not None:
                    drain_tmpl.setdefault(ins.engine, ins)

    seq = [0]

    def _drain_clone(engine, wait):
        tmpl = drain_tmpl[engine]
        d2 = _copy.deepcopy(tmpl)
        seq[0] += 1
        d2.name = f"waitsplit-{seq[0]}"
        d2.sync_info = type(tmpl.sync_info)(on_wait=[wait], on_update=[])
        return d2

    for fn in nc.m.functions:
        for blk in fn.blocks:
            insts = blk.instructions
            idx = 0
            while idx < len(insts):
                ins = insts[idx]
                si = ins.sync_info
                if si and len(si.on_wait) > 1 and ins.engine in drain_tmpl:
                    waits = list(si.on_wait)
                    si.on_wait = waits[-1:]
                    for k, wt in enumerate(waits[:-1]):
                        insts.insert(idx + k, _drain_clone(ins.engine, wt))
                    idx += len(waits) - 1
                idx += 1

    return nc


def _get_nc():
    if "nc" not in _NC_CACHE:
        _NC_CACHE["nc"] = build_nc()
    return _NC_CACHE["nc"]


def run_device(dist8, labf8, trace=False):
    """dist8: [8, W, Q, FI*80]; labf8: [8, P] labels-1 as float."""
    nc = _get_nc()
    np_dt = np.float32 if DIST_DT == "f32" else np.float16
    cls = np.broadcast_to(np.arange(C, dtype=np_dt)[None, :], (Q, C))
    in_maps = []
    for b in range(B):
        labcls = np.concatenate([labf8[b].reshape(Q, 512).astype(np_dt), cls], axis=1)
        in_maps.append(
            {"dist": dist8[b], "labcls": np.ascontiguousarray(labcls)}
        )
    return run_bass_kernel_spmd(nc, in_maps, list(range(B)), trace=trace)


def kernel(
    prototype_distances,
    target_labels,
    proto_class,
    pair_i,
    pair_j,
    pair_cls,
    _trace=False,
    _results_out=None,
):
    dist = np.asarray(prototype_distances, dtype=np.float32).reshape(B, NPROT, P)
    labels = np.asarray(target_labels).reshape(B, P).astype(np.int64)
    proto_class = np.asarray(proto_class, dtype=np.int64)
    pair_i = np.asarray(pair_i, dtype=np.int64)
    pair_j = np.asarray(pair_j, dtype=np.int64)
    pair_cls = np.asarray(pair_cls, dtype=np.int64)

    # Permute prototypes class-major: slot n holds a prototype of class n//8.
    perm = np.empty(NPROT, dtype=np.int64)
    for c in range(C):
        protos = np.nonzero(proto_class == c)[0]
        assert len(protos) == 8, "expect 8 prototypes per class"
        perm[8 * c : 8 * (c + 1)] = protos
    inv = np.empty(NPROT, dtype=np.int64)
    inv[perm] = np.arange(NPROT)

    # Pack into the device DMA layout [B, W, Q, i, n]: pixel p = 512q+FI*w+i.
    np_dt = np.float32 if DIST_DT == "f32" else np.float16
    dist_p = np.ascontiguousarray(
        dist[:, perm, :]
        .reshape(B, NPROT, Q, W, FI)
        .transpose(0, 3, 2, 4, 1)
        .reshape(B, W, Q, FI * NPROT)
        .astype(np_dt)
    )
    labf = np.ascontiguousarray((labels - 1).astype(np_dt))

    br = run_device(dist_p, labf, trace=_trace)
    if _results_out is not None:
        _results_out.append(br)

    total_vals = np.float64(0.0)
    total_valid = 0
    for b in range(B):
        out = br.results[b]["g"]  # [81, 80]; out[j, a] = G[a, j], out[80, a] = Z_a
        Z = out[NPROT].astype(np.float64)
        Gt = out[:NPROT].astype(np.float64)  # Gt[j, a] = sum_p em_a * s_j
        with np.errstate(divide="ignore", invalid="ignore"):
            A = np.where(Z[None, :] != 0.0, Gt / Z[None, :], 0.0)  # A[j, a] = E_a[d_j]
        lb = labels[b] - 1
        cnt = np.bincount(lb[lb >= 0], minlength=C)
        ii = inv[pair_i]
        jj = inv[pair_j]
        # A[x, a] = expectation of d_x under softmax of proto a
        kld = 0.5 * (A[jj, jj] - A[jj, ii] + A[ii, ii] - A[ii, jj])
        valid = cnt[pair_cls] >= 2
        total_vals += np.exp(-kld[valid]).sum()
        total_valid += int(valid.sum())

    if total_valid > 0:
        res = np.float32(total_vals / max(total_valid, 1))
    else:
        res = np.float32(0.0)
    return res


if __name__ == "__main__":
    rng = np.random.default_rng(0)
    d = rng.standard_normal((B, NPROT, 256, 256), dtype=np.float32)
    l = rng.integers(0, 11, (B, 256, 256))
    pc = (np.arange(NPROT) % 40) // 4
    pairs = []
    for s in range(2):
        for c in range(C):
            base = s * 40 + c * 4
            for a in range(4):
                for b2 in range(a + 1, 4):
                    pairs.append((base + a, base + b2, c))
    pairs = np.asarray(pairs, np.int32)
    print(kernel(d, l, pc, pairs[:, 0], pairs[:, 1], pairs[:, 2]))


# revision 20
# speedup vs baseline: 2.7702x; 1.1865x over previous
"""Trainium2 Bass kernel for nn_KLDLoss_18769007083961.

Math reformulation (validated vs reference, rel err ~1e-6):
  For each image b, prototype a with class c(a), define over pixels p:
    s_a[p]  = d_a[p] + (label[p] != c(a)) * (-1e4)      # masked-biased distance
    em_a[p] = exp(s_a[p])                               # exactly 0 off-class (underflow)
    Z_a     = sum_p em_a[p]
    G[a,j]  = sum_p em_a[p] * s_j[p]   (j in same class => same mask)
    A[a,j]  = G[a,j] / Z_a
  Symmetric KL for a same-class pair (i,j) (log-partition terms cancel):
    kld = 0.5 * (A[j,j] - A[j,i] + A[i,i] - A[i,j])
  loss = mean over valid pairs (class count >= 2) of exp(-kld).

Device kernel (one image per NeuronCore, 8 cores):
  Pixel p = 512*q + 64*w + i (q = SBUF partition, w = window, i = inner).
  The host pre-packs dist into [W, Q, FI*80] (i-major: per partition line,
  FI pixel-slots of 80 protos each, class-major proto permutation) so each
  window DMA is one contiguous run per partition (line-rate HBM) and each
  matmul operand slice is contiguous in SBUF.
  Per window: DVE builds the -1e4 class bias into an fp16 s tile (10 STT
  ops, one per class block of 8 protos; s has 81 slots per pixel, slot 80
  memset to 1.0 for the Z row), ACT computes em = exp(s) in fp16, then FI
  fp16 matmuls (lhsT = s-slice [128,81] contiguous, rhs = em-slice
  [128,80] contiguous) accumulate out[m,n] = sum_p s_m * em_n into PSUM
  [81,80]: out[j,a] = G[a,j], row 80 = Z.  Host does the tiny 120-pair
  combination.
"""

import sys
from contextlib import ExitStack

import numpy as np

sys.path.insert(0, "/opt/trn_rl_repo")

import concourse.bass as bass
import concourse.tile as tile
from concourse import mybir
from concourse.bass_utils import run_bass_kernel_spmd
from concourse.tile import add_dep_helper

B = 8
C = 10
NPROT = 80
P = 65536
Q = 128          # partitions = coarse pixel blocks of 512
W = 8            # windows per image
FI = 512 // W    # inner pixels per window per partition
F32 = mybir.dt.float32
F16 = mybir.dt.float16

# dtype of the dist tensor as uploaded to HBM ("f32" or "f16")
DIST_DT = "f32"
# classes whose bias-STT runs on GPSIMD instead of DVE (load balancing)
GPSIMD_CLASSES = ()

_NC_CACHE = {}


def build_nc():
    nc = bass.Bass()
    dt_in = F32 if DIST_DT == "f32" else F16
    d_in = nc.dram_tensor("dist", [W, Q, FI * NPROT], dt_in, kind="ExternalInput")
    # labels [q, 512] packed with the 10 class constants -> cols 512..521
    lab_in = nc.dram_tensor("labcls", [Q, 512 + C], dt_in, kind="ExternalInput")
    g_out = nc.dram_tensor("g", [NPROT + 1, NPROT], F32, kind="ExternalOutput")

    with ExitStack() as ctx:
        tc = ctx.enter_context(tile.TileContext(nc))
        singles = ctx.enter_context(tc.tile_pool(name="singles", bufs=1))
        dpool = ctx.enter_context(tc.tile_pool(name="dpool", bufs=3))
        spool = ctx.enter_context(tc.tile_pool(name="spool", bufs=3))
        empool = ctx.enter_context(tc.tile_pool(name="empool", bufs=3))
        mpool = ctx.enter_context(tc.tile_pool(name="mpool", bufs=2))
        psum = ctx.enter_context(tc.tile_pool(name="psum", bufs=1, space="PSUM"))

        labels_t = singles.tile([Q, 512 + C], dt_in)
        nc.sync.dma_start(out=labels_t, in_=lab_in[:, :])
        cls_t = labels_t[:, 512 : 512 + C]

        g_ps = psum.tile([NPROT + 1, NPROT], F32)

        first = True
        SB = 3  # spool/empool bufs
        em_tiles = []
        exps = []
        # The first dpool.bufs windows go to fresh buffers -> plain SP DMAs
        # with no WAR waits.  Later windows recycle buffers; their DMAs are
        # issued from the ACT sequencer right after exp(w - bufs), whose
        # clock has already observed the DVE ticks of the old buffer's
        # readers, leaving no waits.
        DB = 3  # dpool bufs
        d_tiles = []
        for w in range(DB):
            d_t = dpool.tile([Q, FI * NPROT], dt_in, tag="d", name=f"d_t{w}")
            nc.sync.dma_start(out=d_t, in_=d_in[w])
            d_tiles.append(d_t)
        for w in range(W):
            d_t = d_tiles[w]

            # mne[p, c, i] = (labels != c) as 1.0/0.0
            mne = mpool.tile([Q, C * FI], dt_in, tag="mne")
            lab_w = labels_t[:, w * FI : (w + 1) * FI]
            nc.vector.tensor_tensor(
                mne.rearrange("p (c i) -> p c i", c=C),
                lab_w.unsqueeze(1).broadcast_to([Q, C, FI]),
                cls_t.unsqueeze(2).broadcast_to([Q, C, FI]),
                mybir.AluOpType.not_equal,
            )

            # Absorb the d-DMA tick into DVE's clock with a 1-element copy so
            # the STTs below carry no DMA wait.
            probe = mpool.tile([Q, 1], dt_in, tag="probe", bufs=4)
            nc.vector.tensor_copy(probe, d_t[:, 0:1])
            if w >= SB:
                # s/em buffer recycling gives this window's writers WAR deps
                # on ACT (exp read of s(w-SB)) and PE (matmul reads of s/em
                # (w-SB)).  Absorb each with a 1-element read: em_old[0:1]
                # observes the ACT exp tick; the rotating dummy cell written
                # by i_pabs1(w-SB+1) (which follows all w-SB matmuls in PE
                # order, but is issued early in its window) observes the PE
                # reads without serializing behind newer matmuls.
                em_old = em_tiles[w - SB]
                probe2 = mpool.tile([Q, 1], dt_in, tag="probe2", bufs=4)
                nc.vector.tensor_copy(probe2, em_old[:, 0:1])
                probe3 = mpool.tile([1, 1], F32, tag="probe3", bufs=4)
                nc.vector.tensor_copy(probe3, dummy1[(w - SB + 1) % SB][0:1, 0:1])

            # s[p, i, n] = (mne * -1e4) + d, fp16, one op per class block
            s_t = spool.tile([Q, FI * (NPROT + 1)], F16, tag="s")
            mne_v = mne.rearrange("p (c i) -> p c i", c=C)
            d_v = d_t.rearrange("p (i n) -> p i n", n=NPROT)
            s_v = s_t.rearrange("p (i n) -> p i n", n=NPROT + 1)
            for c in range(C):
                eng = nc.gpsimd if c in GPSIMD_CLASSES else nc.vector
                eng.scalar_tensor_tensor(
                    s_v[:, :, 8 * c : 8 * (c + 1)],
                    mne_v[:, c].unsqueeze(2).broadcast_to([Q, FI, 8]),
                    -1.0e4,
                    d_v[:, :, 8 * c : 8 * (c + 1)],
                    mybir.AluOpType.mult,
                    mybir.AluOpType.add,
                )
            # ones slot (n = 80 of each pixel block) feeds the Z row
            nc.vector.memset(s_v[:, :, NPROT : NPROT + 1], 1.0)

            # ACT-side absorbers so exp(w) carries a single wait: the
            # rotating dummy cell written by i_pabs2(w-SB+1) observes the PE
            # reads of em(w-SB); the ones column observes the DVE tick.
            act_absorbers = []
            if w >= SB:
                dead_act = mpool.tile([1, 1], F32, tag="dead_act", bufs=4)
                act_absorbers.append(
                    nc.scalar.copy(dead_act, dummy2[(w - SB + 1) % SB][0:1, 0:1])
                )
                dead_act3 = mpool.tile([Q, 1], F16, tag="dead_act3", bufs=4)
                act_absorbers.append(
                    nc.scalar.copy(dead_act3, s_t[:, NPROT : NPROT + 1])
                )

            # em = exp(s), including the ones slot (exp(1) unused by rhs)
            em_t = empool.tile([Q, FI * (NPROT + 1)], F16, tag="em")
            em_tiles.append(em_t)
            i_exp = nc.scalar.activation(
                em_t, s_t, mybir.ActivationFunctionType.Exp
            )
            for a in act_absorbers:
                add_dep_helper(i_exp.ins, a.ins, sync=False)
            if w > 0:
                add_dep_helper(i_exp.ins, exps[-1].ins, sync=False)
            exps.append(i_exp)

            if w + DB < W:
                d_next = dpool.tile(
                    [Q, FI * NPROT], dt_in, tag="d", name=f"d_t{w+DB}"
                )
                i_dma = nc.scalar.dma_start(out=d_next, in_=d_in[w + DB])
                add_dep_helper(i_dma.ins, i_exp.ins, sync=False)
                d_tiles.append(d_next)

            # PE-side absorbers: 1x1 dummy matmuls acquire the DVE tick
            # (ones col of s) and the ACT tick (em) so the real matmuls
            # carry no waits.  Their rotating dummy PSUM cells double as
            # "window w matmuls reached" markers for later absorbers.
            ones_col = s_t[:, NPROT : NPROT + 1]
            if w == 0:
                dummy1 = [
                    psum.tile([1, 1], F32, tag=f"dummy1_{k}", bufs=1,
                              name=f"dummy1_{k}")
                    for k in range(SB)
                ]
                dummy2 = [
                    psum.tile([1, 1], F32, tag=f"dummy2_{k}", bufs=1,
                              name=f"dummy2_{k}")
                    for k in range(SB)
                ]
            i_pabs1 = nc.tensor.matmul(
                dummy1[w % SB], ones_col, ones_col,
                start=(w < SB), stop=(w >= W - SB),
                skip_group_check=True,
            )
            i_pabs2 = nc.tensor.matmul(
                dummy2[w % SB], ones_col, em_t[:, 0:1],
                start=(w < SB), stop=(w >= W - SB),
                skip_group_check=True,
            )
            add_dep_helper(i_pabs2.ins, i_pabs1.ins, sync=False)

            for i in range(FI):
                i_mm = nc.tensor.matmul(
                    g_ps,
                    s_t[:, i * (NPROT + 1) : (i + 1) * (NPROT + 1)],
                    em_t[:, i * (NPROT + 1) : i * (NPROT + 1) + NPROT],
                    start=first,
                    stop=(w == W - 1 and i == FI - 1),
                )
                if i == 0:
                    add_dep_helper(i_mm.ins, i_pabs2.ins, sync=False)
                first = False

        g_sb = singles.tile([NPROT + 1, NPROT], F32)
        nc.vector.tensor_copy(g_sb, g_ps)
        nc.sync.dma_start(out=g_out[:, :], in_=g_sb)

    # Hardware instruction structs hold only one sync wait.  Move any excess
    # waits onto single-wait InstDrains injected just before the instruction
    # on the same engine queue (the union of waits still precedes execution).
    import copy as _copy

    drain_tmpl = {}
    for fn in nc.m.functions:
        for blk in fn.blocks:
            for ins in blk.instructions:
                if type(ins).__name__ == "InstDrain" and ins.engine is not None:
                    drain_tmpl.setdefault(ins.engine, ins)

    seq = [0]

    def _drain_clone(engine, wait):
        tmpl = drain_tmpl[engine]
        d2 = _copy.deepcopy(tmpl)
        seq[0] += 1
        d2.name = f"waitsplit-{seq[0]}"
        d2.sync_info = type(tmpl.sync_info)(on_wait=[wait], on_update=[])
        return d2

    for fn in nc.m.functions:
        for blk in fn.blocks:
            insts = blk.instructions
            idx = 0
            while idx < len(insts):
                ins = insts[idx]
                si = ins.sync_info
                if si and len(si.on_wait) > 1 and ins.engine in drain_tmpl:
                    waits = list(si.on_wait)
                    si.on_wait = waits[-1:]
                    for k, wt in enumerate(waits[:-1]):
                        insts.insert(idx + k, _drain_clone(ins.engine, wt))
                    idx += len(waits) - 1
                idx += 1

    return nc


def _get_nc():
    if "nc" not in _NC_CACHE:
        _NC_CACHE["nc"] = build_nc()
    return _NC_CACHE["nc"]


def run_device(dist8, labf8, trace=False):
    """dist8: [8, W, Q, FI*80]; labf8: [8, P] labels-1 as float."""
    nc = _get_nc()
    np_dt = np.float32 if DIST_DT == "f32" else np.float16
    cls = np.broadcast_to(np.arange(C, dtype=np_dt)[None, :], (Q, C))
    in_maps = []
    for b in range(B):
        labcls = np.concatenate([labf8[b].reshape(Q, 512).astype(np_dt), cls], axis=1)
        in_maps.append(
            {"dist": dist8[b], "labcls": np.ascontiguousarray(labcls)}
        )
    return run_bass_kernel_spmd(nc, in_maps, list(range(B)), trace=trace)


def kernel(
    prototype_distances,
    target_labels,
    proto_class,
    pair_i,
    pair_j,
    pair_cls,
    _trace=False,
    _results_out=None,
):
    dist = np.asarray(prototype_distances, dtype=np.float32).reshape(B, NPROT, P)
    labels = np.asarray(target_labels).reshape(B, P).astype(np.int64)
    proto_class = np.asarray(proto_class, dtype=np.int64)
    pair_i = np.asarray(pair_i, dtype=np.int64)
    pair_j = np.asarray(pair_j, dtype=np.int64)
    pair_cls = np.asarray(pair_cls, dtype=np.int64)

    # Permute prototypes class-major: slot n holds a prototype of class n//8.
    perm = np.empty(NPROT, dtype=np.int64)
    for c in range(C):
        protos = np.nonzero(proto_class == c)[0]
        assert len(protos) == 8, "expect 8 prototypes per class"
        perm[8 * c : 8 * (c + 1)] = protos
    inv = np.empty(NPROT, dtype=np.int64)
    inv[perm] = np.arange(NPROT)

    # Pack into the device DMA layout [B, W, Q, i, n]: pixel p = 512q+FI*w+i.
    np_dt = np.float32 if DIST_DT == "f32" else np.float16
    dist_p = np.ascontiguousarray(
        dist[:, perm, :]
        .reshape(B, NPROT, Q, W, FI)
        .transpose(0, 3, 2, 4, 1)
        .reshape(B, W, Q, FI * NPROT)
        .astype(np_dt)
    )
    labf = np.ascontiguousarray((labels - 1).astype(np_dt))

    br = run_device(dist_p, labf, trace=_trace)
    if _results_out is not None:
        _results_out.append(br)

    total_vals = np.float64(0.0)
    total_valid = 0
    for b in range(B):
        out = br.results[b]["g"]  # [81, 80]; out[j, a] = G[a, j], out[80, a] = Z_a
        Z = out[NPROT].astype(np.float64)
        Gt = out[:NPROT].astype(np.float64)  # Gt[j, a] = sum_p em_a * s_j
        with np.errstate(divide="ignore", invalid="ignore"):
            A = np.where(Z[None, :] != 0.0, Gt / Z[None, :], 0.0)  # A[j, a] = E_a[d_j]
        lb = labels[b] - 1
        cnt = np.bincount(lb[lb >= 0], minlength=C)
        ii = inv[pair_i]
        jj = inv[pair_j]
        # A[x, a] = expectation of d_x under softmax of proto a
        kld = 0.5 * (A[jj, jj] - A[jj, ii] + A[ii, ii] - A[ii, jj])
        valid = cnt[pair_cls] >= 2
        total_vals += np.exp(-kld[valid]).sum()
        total_valid += int(valid.sum())

    if total_valid > 0:
        res = np.float32(total_vals / max(total_valid, 1))
    else:
        res = np.float32(0.0)
    return res


if __name__ == "__main__":
    rng = np.random.default_rng(0)
    d = rng.standard_normal((B, NPROT, 256, 256), dtype=np.float32)
    l = rng.integers(0, 11, (B, 256, 256))
    pc = (np.arange(NPROT) % 40) // 4
    pairs = []
    for s in range(2):
        for c in range(C):
            base = s * 40 + c * 4
            for a in range(4):
                for b2 in range(a + 1, 4):
                    pairs.append((base + a, base + b2, c))
    pairs = np.asarray(pairs, np.int32)
    print(kernel(d, l, pc, pairs[:, 0], pairs[:, 1], pairs[:, 2]))


# revision 21
# speedup vs baseline: 3.1128x; 1.1237x over previous
"""Trainium2 Bass kernel for nn_KLDLoss_18769007083961.

Math reformulation (validated vs reference, rel err ~1e-6):
  For each image b, prototype a with class c(a), define over pixels p:
    s_a[p]  = d_a[p] + (label[p] != c(a)) * (-1e4)      # masked-biased distance
    em_a[p] = exp(s_a[p])                               # exactly 0 off-class (underflow)
    Z_a     = sum_p em_a[p]
    G[a,j]  = sum_p em_a[p] * s_j[p]   (j in same class => same mask)
    A[a,j]  = G[a,j] / Z_a
  Symmetric KL for a same-class pair (i,j) (log-partition terms cancel):
    kld = 0.5 * (A[j,j] - A[j,i] + A[i,i] - A[i,j])
  loss = mean over valid pairs (class count >= 2) of exp(-kld).

Device kernel (one image per NeuronCore, 8 cores):
  Pixel p = 512*q + 64*w + i (q = SBUF partition, w = window, i = inner).
  The host pre-packs dist into [W, Q, FI*80] (i-major: per partition line,
  FI pixel-slots of 80 protos each, class-major proto permutation) so each
  window DMA is one contiguous run per partition (line-rate HBM) and each
  matmul operand slice is contiguous in SBUF.
  Per window: DVE builds the -1e4 class bias into an fp16 s tile (10 STT
  ops, one per class block of 8 protos; s has 81 slots per pixel, slot 80
  memset to 1.0 for the Z row), ACT computes em = exp(s) in fp16, then FI
  fp16 matmuls (lhsT = s-slice [128,81] contiguous, rhs = em-slice
  [128,80] contiguous) accumulate out[m,n] = sum_p s_m * em_n into PSUM
  [81,80]: out[j,a] = G[a,j], row 80 = Z.  Host does the tiny 120-pair
  combination.
"""

import sys
from contextlib import ExitStack

import numpy as np

sys.path.insert(0, "/opt/trn_rl_repo")

import concourse.bass as bass
import concourse.tile as tile
from concourse import mybir
from concourse.bass_utils import run_bass_kernel_spmd
from concourse.tile import add_dep_helper

B = 8
C = 10
NPROT = 80
P = 65536
Q = 128          # partitions = coarse pixel blocks of 512
W = 8            # windows per image
FI = 512 // W    # inner pixels per window per partition
F32 = mybir.dt.float32
F16 = mybir.dt.float16

# dtype of the dist tensor as uploaded to HBM ("f32" or "f16")
DIST_DT = "f16"
# classes whose bias-STT runs on GPSIMD instead of DVE (load balancing)
GPSIMD_CLASSES = ()

_NC_CACHE = {}


def build_nc():
    nc = bass.Bass()
    dt_in = F32 if DIST_DT == "f32" else F16
    d_in = nc.dram_tensor("dist", [W, Q, FI * NPROT], dt_in, kind="ExternalInput")
    # labels [q, 512] packed with the 10 class constants -> cols 512..521
    lab_in = nc.dram_tensor("labcls", [Q, 512 + C], dt_in, kind="ExternalInput")
    g_out = nc.dram_tensor("g", [NPROT + 1, NPROT], F32, kind="ExternalOutput")

    with ExitStack() as ctx:
        tc = ctx.enter_context(tile.TileContext(nc))
        singles = ctx.enter_context(tc.tile_pool(name="singles", bufs=1))
        dpool = ctx.enter_context(tc.tile_pool(name="dpool", bufs=3))
        spool = ctx.enter_context(tc.tile_pool(name="spool", bufs=3))
        empool = ctx.enter_context(tc.tile_pool(name="empool", bufs=3))
        mpool = ctx.enter_context(tc.tile_pool(name="mpool", bufs=2))
        psum = ctx.enter_context(tc.tile_pool(name="psum", bufs=1, space="PSUM"))

        labels_t = singles.tile([Q, 512 + C], dt_in)
        nc.sync.dma_start(out=labels_t, in_=lab_in[:, :])
        cls_t = labels_t[:, 512 : 512 + C]

        g_ps = psum.tile([NPROT + 1, NPROT], F32)

        first = True
        SB = 3  # spool/empool bufs
        em_tiles = []
        exps = []
        # The first dpool.bufs windows go to fresh buffers -> plain SP DMAs
        # with no WAR waits.  Later windows recycle buffers; their DMAs are
        # issued from the ACT sequencer right after exp(w - bufs), whose
        # clock has already observed the DVE ticks of the old buffer's
        # readers, leaving no waits.
        DB = 3  # dpool bufs
        d_tiles = []
        for w in range(DB):
            d_t = dpool.tile([Q, FI * NPROT], dt_in, tag="d", name=f"d_t{w}")
            nc.sync.dma_start(out=d_t, in_=d_in[w])
            d_tiles.append(d_t)
        for w in range(W):
            d_t = d_tiles[w]

            # mne[p, c, i] = (labels != c) as 1.0/0.0
            mne = mpool.tile([Q, C * FI], dt_in, tag="mne")
            lab_w = labels_t[:, w * FI : (w + 1) * FI]
            nc.vector.tensor_tensor(
                mne.rearrange("p (c i) -> p c i", c=C),
                lab_w.unsqueeze(1).broadcast_to([Q, C, FI]),
                cls_t.unsqueeze(2).broadcast_to([Q, C, FI]),
                mybir.AluOpType.not_equal,
            )

            # Absorb the d-DMA tick into DVE's clock with a 1-element copy so
            # the STTs below carry no DMA wait.
            probe = mpool.tile([Q, 1], dt_in, tag="probe", bufs=4)
            nc.vector.tensor_copy(probe, d_t[:, 0:1])
            if w >= SB:
                # s/em buffer recycling gives this window's writers WAR deps
                # on ACT (exp read of s(w-SB)) and PE (matmul reads of s/em
                # (w-SB)).  Absorb each with a 1-element read: em_old[0:1]
                # observes the ACT exp tick; the rotating dummy cell written
                # by i_pabs1(w-SB+1) (which follows all w-SB matmuls in PE
                # order, but is issued early in its window) observes the PE
                # reads without serializing behind newer matmuls.
                em_old = em_tiles[w - SB]
                probe2 = mpool.tile([Q, 1], dt_in, tag="probe2", bufs=4)
                nc.vector.tensor_copy(probe2, em_old[:, 0:1])
                probe3 = mpool.tile([1, 1], F32, tag="probe3", bufs=4)
                nc.vector.tensor_copy(probe3, dummy1[(w - SB + 1) % SB][0:1, 0:1])

            # s[p, i, n] = (mne * -1e4) + d, fp16, one op per class block
            s_t = spool.tile([Q, FI * (NPROT + 1)], F16, tag="s")
            mne_v = mne.rearrange("p (c i) -> p c i", c=C)
            d_v = d_t.rearrange("p (i n) -> p i n", n=NPROT)
            s_v = s_t.rearrange("p (i n) -> p i n", n=NPROT + 1)
            for c in range(C):
                eng = nc.gpsimd if c in GPSIMD_CLASSES else nc.vector
                eng.scalar_tensor_tensor(
                    s_v[:, :, 8 * c : 8 * (c + 1)],
                    mne_v[:, c].unsqueeze(2).broadcast_to([Q, FI, 8]),
                    -1.0e4,
                    d_v[:, :, 8 * c : 8 * (c + 1)],
                    mybir.AluOpType.mult,
                    mybir.AluOpType.add,
                )
            # ones slot (n = 80 of each pixel block) feeds the Z row
            nc.vector.memset(s_v[:, :, NPROT : NPROT + 1], 1.0)

            # ACT-side absorbers so exp(w) carries a single wait: the
            # rotating dummy cell written by i_pabs2(w-SB+1) observes the PE
            # reads of em(w-SB); the ones column observes the DVE tick.
            act_absorbers = []
            if w >= SB:
                dead_act = mpool.tile([1, 1], F32, tag="dead_act", bufs=4)
                act_absorbers.append(
                    nc.scalar.copy(dead_act, dummy2[(w - SB + 1) % SB][0:1, 0:1])
                )
                dead_act3 = mpool.tile([Q, 1], F16, tag="dead_act3", bufs=4)
                act_absorbers.append(
                    nc.scalar.copy(dead_act3, s_t[:, NPROT : NPROT + 1])
                )

            # em = exp(s), including the ones slot (exp(1) unused by rhs)
            em_t = empool.tile([Q, FI * (NPROT + 1)], F16, tag="em")
            em_tiles.append(em_t)
            i_exp = nc.scalar.activation(
                em_t, s_t, mybir.ActivationFunctionType.Exp
            )
            for a in act_absorbers:
                add_dep_helper(i_exp.ins, a.ins, sync=False)
            if w > 0:
                add_dep_helper(i_exp.ins, exps[-1].ins, sync=False)
            exps.append(i_exp)

            if w + DB < W:
                d_next = dpool.tile(
                    [Q, FI * NPROT], dt_in, tag="d", name=f"d_t{w+DB}"
                )
                i_dma = nc.scalar.dma_start(out=d_next, in_=d_in[w + DB])
                add_dep_helper(i_dma.ins, i_exp.ins, sync=False)
                d_tiles.append(d_next)

            # PE-side absorbers: 1x1 dummy matmuls acquire the DVE tick
            # (ones col of s) and the ACT tick (em) so the real matmuls
            # carry no waits.  Their rotating dummy PSUM cells double as
            # "window w matmuls reached" markers for later absorbers.
            ones_col = s_t[:, NPROT : NPROT + 1]
            if w == 0:
                dummy1 = [
                    psum.tile([1, 1], F32, tag=f"dummy1_{k}", bufs=1,
                              name=f"dummy1_{k}")
                    for k in range(SB)
                ]
                dummy2 = [
                    psum.tile([1, 1], F32, tag=f"dummy2_{k}", bufs=1,
                              name=f"dummy2_{k}")
                    for k in range(SB)
                ]
            i_pabs1 = nc.tensor.matmul(
                dummy1[w % SB], ones_col, ones_col,
                start=(w < SB), stop=(w >= W - SB),
                skip_group_check=True,
            )
            i_pabs2 = nc.tensor.matmul(
                dummy2[w % SB], ones_col, em_t[:, 0:1],
                start=(w < SB), stop=(w >= W - SB),
                skip_group_check=True,
            )
            add_dep_helper(i_pabs2.ins, i_pabs1.ins, sync=False)

            for i in range(FI):
                i_mm = nc.tensor.matmul(
                    g_ps,
                    s_t[:, i * (NPROT + 1) : (i + 1) * (NPROT + 1)],
                    em_t[:, i * (NPROT + 1) : i * (NPROT + 1) + NPROT],
                    start=first,
                    stop=(w == W - 1 and i == FI - 1),
                )
                if i == 0:
                    add_dep_helper(i_mm.ins, i_pabs2.ins, sync=False)
                first = False

        g_sb = singles.tile([NPROT + 1, NPROT], F32)
        nc.vector.tensor_copy(g_sb, g_ps)
        nc.sync.dma_start(out=g_out[:, :], in_=g_sb)

    # Hardware instruction structs hold only one sync wait.  Move any excess
    # waits onto single-wait InstDrains injected just before the instruction
    # on the same engine queue (the union of waits still precedes execution).
    import copy as _copy

    drain_tmpl = {}
    for fn in nc.m.functions:
        for blk in fn.blocks:
            for ins in blk.instructions:
                if type(ins).__name__ == "InstDrain" and ins.engine is not None:
                    drain_tmpl.setdefault(ins.engine, ins)

    seq = [0]

    def _drain_clone(engine, wait):
        tmpl = drain_tmpl[engine]
        d2 = _copy.deepcopy(tmpl)
        seq[0] += 1
        d2.name = f"waitsplit-{seq[0]}"
        d2.sync_info = type(tmpl.sync_info)(on_wait=[wait], on_update=[])
        return d2

    for fn in nc.m.functions:
        for blk in fn.blocks:
            insts = blk.instructions
            idx = 0
            while idx < len(insts):
                ins = insts[idx]
                si = ins.sync_info
                if si and len(si.on_wait) > 1 and ins.engine in drain_tmpl:
                    waits = list(si.on_wait)
                    si.on_wait = waits[-1:]
                    for k, wt in enumerate(waits[:-1]):
                        insts.insert(idx + k, _drain_clone(ins.engine, wt))
                    idx += len(waits) - 1
                idx += 1

    return nc


def _get_nc():
    if "nc" not in _NC_CACHE:
        _NC_CACHE["nc"] = build_nc()
    return _NC_CACHE["nc"]


def run_device(dist8, labf8, trace=False):
    """dist8: [8, W, Q, FI*80]; labf8: [8, P] labels-1 as float."""
    nc = _get_nc()
    np_dt = np.float32 if DIST_DT == "f32" else np.float16
    cls = np.broadcast_to(np.arange(C, dtype=np_dt)[None, :], (Q, C))
    in_maps = []
    for b in range(B):
        labcls = np.concatenate([labf8[b].reshape(Q, 512).astype(np_dt), cls], axis=1)
        in_maps.append(
            {"dist": dist8[b], "labcls": np.ascontiguousarray(labcls)}
        )
    return run_bass_kernel_spmd(nc, in_maps, list(range(B)), trace=trace)


def kernel(
    prototype_distances,
    target_labels,
    proto_class,
    pair_i,
    pair_j,
    pair_cls,
    _trace=False,
    _results_out=None,
):
    dist = np.asarray(prototype_distances, dtype=np.float32).reshape(B, NPROT, P)
    labels = np.asarray(target_labels).reshape(B, P).astype(np.int64)
    proto_class = np.asarray(proto_class, dtype=np.int64)
    pair_i = np.asarray(pair_i, dtype=np.int64)
    pair_j = np.asarray(pair_j, dtype=np.int64)
    pair_cls = np.asarray(pair_cls, dtype=np.int64)

    # Permute prototypes class-major: slot n holds a prototype of class n//8.
    perm = np.empty(NPROT, dtype=np.int64)
    for c in range(C):
        protos = np.nonzero(proto_class == c)[0]
        assert len(protos) == 8, "expect 8 prototypes per class"
        perm[8 * c : 8 * (c + 1)] = protos
    inv = np.empty(NPROT, dtype=np.int64)
    inv[perm] = np.arange(NPROT)

    # Pack into the device DMA layout [B, W, Q, i, n]: pixel p = 512q+FI*w+i.
    np_dt = np.float32 if DIST_DT == "f32" else np.float16
    dist_p = np.ascontiguousarray(
        dist[:, perm, :]
        .reshape(B, NPROT, Q, W, FI)
        .transpose(0, 3, 2, 4, 1)
        .reshape(B, W, Q, FI * NPROT)
        .astype(np_dt)
    )
    labf = np.ascontiguousarray((labels - 1).astype(np_dt))

    br = run_device(dist_p, labf, trace=_trace)
    if _results_out is not None:
        _results_out.append(br)

    total_vals = np.float64(0.0)
    total_valid = 0
    for b in range(B):
        out = br.results[b]["g"]  # [81, 80]; out[j, a] = G[a, j], out[80, a] = Z_a
        Z = out[NPROT].astype(np.float64)
        Gt = out[:NPROT].astype(np.float64)  # Gt[j, a] = sum_p em_a * s_j
        with np.errstate(divide="ignore", invalid="ignore"):
            A = np.where(Z[None, :] != 0.0, Gt / Z[None, :], 0.0)  # A[j, a] = E_a[d_j]
        lb = labels[b] - 1
        cnt = np.bincount(lb[lb >= 0], minlength=C)
        ii = inv[pair_i]
        jj = inv[pair_j]
        # A[x, a] = expectation of d_x under softmax of proto a
        kld = 0.5 * (A[jj, jj] - A[jj, ii] + A[ii, ii] - A[ii, jj])
        valid = cnt[pair_cls] >= 2
        total_vals += np.exp(-kld[valid]).sum()
        total_valid += int(valid.sum())

    if total_valid > 0:
        res = np.float32(total_vals / max(total_valid, 1))
    else:
        res = np.float32(0.0)
    return res


if __name__ == "__main__":
    rng = np.random.default_rng(0)
    d = rng.standard_normal((B, NPROT, 256, 256), dtype=np.float32)
    l = rng.integers(0, 11, (B, 256, 256))
    pc = (np.arange(NPROT) % 40) // 4
    pairs = []
    for s in range(2):
        for c in range(C):
            base = s * 40 + c * 4
            for a in range(4):
                for b2 in range(a + 1, 4):
                    pairs.append((base + a, base + b2, c))
    pairs = np.asarray(pairs, np.int32)
    print(kernel(d, l, pc, pairs[:, 0], pairs[:, 1], pairs[:, 2]))
